# revision 1
# baseline (speedup 1.0000x reference)
"""DiffuseRouter kernel for 8 TRN2 NeuronCores.

Reference computation (enable_time=False, soft_time_routing=True):
    out[b, l, d] = (1/3) * sum_g sum_e expert_emb_g[e, b, l, d]
i.e. a uniform-weighted sum of 28 expert planes per batch element.

Sharding: pure data-parallel over batch B=8 -> one batch element per core.
Each core reads its 28 [256, 1280] f32 planes (36.7 MB), reduces them
on-chip, scales by 1/3, and writes its [256, 1280] output.  No collectives
needed (B == n_cores), which is strictly less traffic than expert-parallel
+ all-reduce.
"""

import numpy as np

import concourse.bacc as bacc
import concourse.tile as tile
from concourse import mybir
from concourse.alu_op_type import AluOpType
from concourse.bass_utils import run_bass_kernel_spmd

N_CORES = 8
E_TOTAL = 28  # 4 + 8 + 16 experts across the 3 granularity levels
L, D = 256, 1280
P = 128  # SBUF partitions
FD = (L // P) * D  # 2560 free-dim elements per partition
SCALE = 1.0 / 3.0

_NC_CACHE = None


def _build_nc():
    """Build the SPMD Bass program (identical on all 8 cores).

    Structure: stream the 28 expert planes as [128, 2560] tiles (1.31 MB
    linear DMAs) on the SP HWDGE ring; accumulate on DVE in two independent
    half-chains over the free dim (cols [0:1280) and [1280:2560)) with the
    1/3 scale folded into every add via scalar_tensor_tensor, so each half
    can be stored the moment its last add retires.  Stores go on the ACT
    HWDGE ring so they never queue behind input loads.
    """
    nc = bacc.Bacc(
        "TRN2", target_bir_lowering=False, debug=False, enable_partition_id=False
    )
    x = nc.dram_tensor("x", [E_TOTAL, L, D], mybir.dt.float32, kind="ExternalInput")
    out = nc.dram_tensor("out", [L, D], mybir.dt.float32, kind="ExternalOutput")

    # [E, 256, 1280] -> [E, 128, 2560]: partition p holds rows 2p, 2p+1
    # (contiguous 10240 B per partition -> fully linear 1.31 MB DMA per plane).
    x_t = x.ap().rearrange("e (p a) d -> e p (a d)", a=2)
    out_t = out.ap().rearrange("(p a) d -> p (a d)", a=2)

    H = FD // 2  # half of the free dim
    halves = [slice(0, H), slice(H, FD)]
    mult = AluOpType.mult
    add = AluOpType.add

    with tile.TileContext(nc) as tc:
        with (
            tc.tile_pool(name="in", bufs=8) as pin,
            tc.tile_pool(name="acc", bufs=2) as pacc,
        ):
            accs = [
                pacc.tile([P, H], mybir.dt.float32, name=f"acc{i}", tag=f"acc{i}")
                for i in range(2)
            ]
            last = E_TOTAL - 1
            for e in range(E_TOTAL):
                if e < last:
                    # All input loads on the SP HWDGE ring: strict FIFO order
                    # matches the accumulation order, so exactly one tile's
                    # adds remain after the stream ends.
                    t = pin.tile([P, FD], mybir.dt.float32)
                    nc.sync.dma_start(out=t[:], in_=x_t[e])
                    ths = [t[:, h] for h in halves]
                else:
                    # Last expert: four quarter-loads in separate tiles so
                    # each final quarter-add starts as soon as its own
                    # quarter lands (not its half).
                    Q = FD // 4
                    qts = []
                    for qi in range(4):
                        qt = pin.tile(
                            [P, Q], mybir.dt.float32, name=f"tq{qi}", tag=f"tq{qi}"
                        )
                        nc.sync.dma_start(
                            out=qt[:], in_=x_t[e][:, qi * Q : (qi + 1) * Q]
                        )
                        qts.append(qt[:])
                    ths = qts
                if e < last:
                    for acc, th in zip(accs, ths):
                        if e == 0:
                            # acc = t0 * 1/3 (tensor_scalar: 2x perf mode)
                            nc.vector.tensor_scalar_mul(acc[:], th, SCALE)
                        else:
                            # acc = (t_e * 1/3) + acc
                            nc.vector.scalar_tensor_tensor(
                                acc[:], th, SCALE, acc[:], mult, add
                            )
                else:
                    # Final adds split into quarters so each quarter-store
                    # can fire as soon as its own quarter retires.
                    Q = H // 2
                    for qi in range(4):
                        acc = accs[qi // 2]
                        q = slice((qi % 2) * Q, (qi % 2 + 1) * Q)
                        nc.vector.scalar_tensor_tensor(
                            acc[:, q], ths[qi], SCALE, acc[:, q], mult, add
                        )
            # Quarter-stores alternating rings per quarter (ACT, SP, ACT, SP)
            # so consecutive quarters never queue behind each other in one
            # ring's FIFO — the last quarter's store issues immediately.
            Q = H // 2
            for hi, acc in enumerate(accs):
                for qi in range(2):
                    q = slice(qi * Q, (qi + 1) * Q)
                    gq = slice(hi * H + qi * Q, hi * H + (qi + 1) * Q)
                    eng = nc.scalar if (hi * 2 + qi) % 2 == 0 else nc.sync
                    eng.dma_start(out=out_t[:, gq], in_=acc[:, q])
    nc.compile()
    return nc


def _get_nc():
    global _NC_CACHE
    if _NC_CACHE is None:
        _NC_CACHE = _build_nc()
    return _NC_CACHE


def _run(inputs, trace=False, trace_kwargs=None):
    e0 = np.asarray(inputs["expert_emb_0"], dtype=np.float32)
    e1 = np.asarray(inputs["expert_emb_1"], dtype=np.float32)
    e2 = np.asarray(inputs["expert_emb_2"], dtype=np.float32)
    B = e0.shape[1]
    assert B == N_CORES, f"expected B == {N_CORES}, got {B}"

    in_maps = []
    for b in range(B):
        xb = np.concatenate([e0[:, b], e1[:, b], e2[:, b]], axis=0)
        in_maps.append({"x": np.ascontiguousarray(xb)})

    kw = {}
    if trace:
        kw["trace"] = True
        if trace_kwargs:
            kw.update(trace_kwargs)
    try:
        res = run_bass_kernel_spmd(_get_nc(), in_maps, list(range(N_CORES)), **kw)
    except Exception:
        # One retry: transient device errors (e.g. NRT unrecoverable after a
        # prior wedged run) usually clear on re-dispatch.
        res = run_bass_kernel_spmd(_get_nc(), in_maps, list(range(N_CORES)), **kw)
    out = np.stack([res.results[b]["out"] for b in range(B)], axis=0)
    return out.astype(np.float32, copy=False), res


def kernel(**inputs) -> np.ndarray:
    out, _ = _run(inputs, trace=False)
    return out

